# revision 22
# baseline (speedup 1.0000x reference)
"""CV neural network (6 modes, cutoff 3, 6 layers) on 8 trn2 NeuronCores.

Algebra: the reference circuit is
    psi0(x_b) = kron_m expm(x_bm * D_GEN)[:, 0]          (closed form, host)
    psi       = C @ psi0                                  (C fixed 729x729)
    out[b,m]  = Re( psi^H (I (x) X_OP (x) I) psi )        (host)
Everything between the data-encoding displacements and the expectations is a
fixed linear operator C on the 729-dim truncated Fock space, depending only on
the (tiny) layer parameters.  The host folds the circuit into UT = C^T once
(complex128); the device does the heavy part: psi[b,:] = psi0[b,:] @ UT.

Key structure: D_GEN = A^T - A is REAL, so the encoding displacement gates and
hence psi0 are REAL vectors.  The device matmul is therefore real x complex:
    re = P @ Ur,  im = P @ Ui            (2 real matmuls, not 4)
and with the loose 2e-2 rel-err budget all operands ride in bf16 (measured
end-to-end rel err ~1.6e-3), halving DMA bytes again vs f32.

Sharding: batch 4-way x output-column (i) 2-way = 8 cores.  Per core:
  p_ri: [128, 6*256]   psi0^T batch-quarter, bf16, K-blocked (j = 128*jt + p)
  u_ri: [128, 6*732]   UT column half, bf16, K-blocked, each block re|im
  o_ri: [256, 732]     psi shard bf16 (b rows; cols re|im)
Five DMAs total (p, u in 3 chunks, out x2), all on the two HWDGE rings; no
SWDGE, no scalar-engine compute (avoids its ACT table load).
"""
import os
import numpy as np

N_MODES, N_LAYERS, CUTOFF, BATCH = 6, 6, 3, 1024
M2 = N_MODES * (N_MODES - 1) // 2
DIM = CUTOFF ** N_MODES                      # 729
N_CORES = 8
B_SHARD = BATCH // 4                         # 256 (batch quarter)
I_SHARD = 366                                # column half; overlap 3
I_START = (0, DIM - I_SHARD)                 # (0, 363)
DIM_PAD = 768                                # 6 x 128 (rows 729.. are zero)
NJ = 6                                       # j tiles, K=128 (last 89 live)

# Results of the last device run (for the test harness to inspect).
LAST_RESULT = None

# ----------------------------------------------------------------- host math

_a = np.diag(np.sqrt(np.arange(1, CUTOFF)), 1).astype(np.complex128)
_ad = _a.conj().T
_NVEC = np.arange(CUTOFF, dtype=np.float64)
_X_OP = (_a + _ad).real
_BS_GEN = np.kron(_ad, _a) - np.kron(_a, _ad)
_SQ_GEN = _a @ _a - _ad @ _ad
_D_GEN = _ad - _a


def _expm_factory(G):
    """G anti-Hermitian. Returns f(t) = expm(t*G), vectorized over real t."""
    lam, V = np.linalg.eigh(1j * G)
    Vh = V.conj().T

    def f(t):
        t = np.asarray(t, dtype=np.float64)
        ph = np.exp(-1j * np.multiply.outer(t, lam))
        return np.einsum('ij,...j,jk->...ik', V, ph, Vh)
    return f


_disp_gate = _expm_factory(_D_GEN)
_sq_gate_half = _expm_factory(0.5 * _SQ_GEN)
_bs_gate = _expm_factory(_BS_GEN)


def _apply_1(psi, U, m):
    psi = np.moveaxis(psi, 1 + m, -1)
    psi = psi @ U.T
    return np.moveaxis(psi, -1, 1 + m)


def _apply_2(psi, U, m):
    psi = np.moveaxis(psi, (1 + m, 2 + m), (-2, -1))
    sh = psi.shape
    psi = (psi.reshape(sh[:-2] + (CUTOFF * CUTOFF,)) @ U.T).reshape(sh)
    return np.moveaxis(psi, (-2, -1), (1 + m, 2 + m))


def _apply_diag(psi, d, m):
    shape = [1] * psi.ndim
    shape[1 + m] = CUTOFF
    return psi * d.reshape(shape)


def _interferometer(psi, params):
    theta = params[:M2]
    rphi = params[-N_MODES:]
    n = 0
    for l in range(N_MODES):
        for k in range(N_MODES - 1):
            if (l + k) % 2 != 1:
                psi = _apply_2(psi, _bs_gate(theta[n]), k)
                n += 1
    for i in range(max(1, N_MODES - 1)):
        psi = _apply_diag(psi, np.exp(1j * rphi[i] * _NVEC), i)
    return psi


def _build_UT(theta_1, theta_2, squeezing_r, displacement_r, kerr_params):
    """UT[j, i] = C[i, j]: apply the post-encoding circuit to basis vectors."""
    psi = np.eye(DIM, dtype=np.complex128).reshape((DIM,) + (CUTOFF,) * N_MODES)
    for L in range(N_LAYERS):
        psi = _interferometer(psi, theta_1[L])
        for m in range(N_MODES):
            psi = _apply_1(psi, _sq_gate_half(squeezing_r[L, m] * 0.5), m)
        psi = _interferometer(psi, theta_2[L])
        for m in range(N_MODES):
            psi = _apply_1(psi, _disp_gate(displacement_r[L, m]), m)
            psi = _apply_diag(
                psi, np.exp(1j * (kerr_params[L, m] * 0.001) * _NVEC * _NVEC), m)
    return psi.reshape(DIM, DIM)


def _build_psi0(x):
    """x: (B, 6) -> flattened kron of displacement columns, (B, 729) real."""
    v = _disp_gate(x)[..., :, 0].real
    out = v[:, 0, :]
    for m in range(1, N_MODES):
        out = np.einsum('bi,bj->bij', out, v[:, m, :]).reshape(x.shape[0], -1)
    return out


def _expectation(psi_flat):
    """psi_flat: (B, 729) complex -> (B, 6) float64: <X_m>."""
    B = psi_flat.shape[0]
    outs = []
    for m in range(N_MODES):
        pre, post = CUTOFF ** m, CUTOFF ** (N_MODES - 1 - m)
        psi = psi_flat.reshape(B, pre, CUTOFF, post)
        phi = np.einsum('ij,bpjq->bpiq', _X_OP, psi)
        outs.append(np.sum(psi.conj() * phi, axis=(1, 2, 3)).real)
    return np.stack(outs, axis=1)


# --------------------------------------------------------------- bass kernel

def _build_bass():
    import concourse.mybir as mybir
    import concourse.tile as tile
    from concourse import bacc

    nc = bacc.Bacc("TRN2", target_bir_lowering=False, debug=False,
                   enable_asserts=False, num_devices=N_CORES)
    f32 = mybir.dt.float32
    bf16 = mybir.dt.bfloat16

    UB = 2 * I_SHARD                               # 732: re|im per j block
    CB = B_SHARD + UB                              # 988 cols per jt bundle
    c_ri = nc.dram_tensor("c_ri", [128, NJ * CB], bf16,
                          kind="ExternalInput").ap()
    o_ri = nc.dram_tensor("o_ri", [B_SHARD, UB], bf16,
                          kind="ExternalOutput").ap()

    with tile.TileContext(nc) as tc:
        with (
            tc.tile_pool(name="c", bufs=6) as c_pool,
            tc.tile_pool(name="ps", bufs=2, space="PSUM") as ps_pool,
            tc.tile_pool(name="o", bufs=2) as o_pool,
            tc.tile_pool(name="s", bufs=1) as s_pool,
        ):
            # Bundles [p jt | u_re jt | u_im jt].  Ring facts (measured):
            # each HWDGE ring moves ~160 GB/s with >=2.9KB/partition
            # descriptors but only ~110-130 with ~2KB ones; rings drain
            # FIFO; chunks on different rings round-robin and complete
            # together.  So: small single-jt chunks first on each ring for
            # an early start, big 2-jt chunks behind them for rate; equal
            # bytes per ring.  128 rows always (an 89-row DMA degenerates to
            # a serial ~77ns/descriptor trickle).
            # Warm-tile memset first in the gpsimd stream so PE warm-up can
            # start at ~7.4us (vector's first op otherwise lands ~1us late).
            wsrc = s_pool.tile([128, 256], bf16, tag="warm", name="warm")
            nc.gpsimd.memset(wsrc[:, :], 0)

            groups = [((0,), nc.sync), ((1,), nc.scalar),
                      ((2, 3), nc.sync), ((4, 5), nc.scalar)]
            c_t = {}
            for g, (jts, eng) in enumerate(groups):
                t = c_pool.tile([128, len(jts) * CB], bf16, tag="c",
                                name=f"c{g}")
                eng.dma_start(
                    out=t, in_=c_ri[:, jts[0] * CB:(jts[-1] + 1) * CB])
                for i, jt in enumerate(jts):
                    c_t[jt] = t[:, i * CB:(i + 1) * CB]

            # PE warm-up: back-to-back dummy bf16 matmuls (~100% PE duty)
            # bridge the input-load window; the HAM un-throttles the PE
            # (1.2 -> 2.4 GHz) only after ~3.8us of gapless activity, and any
            # idle gap restarts it cold.
            ps_w = ps_pool.tile([128, 512], f32, tag="psw", name="psw", bufs=1)

            def warm(n):
                for w in range(n):
                    nc.tensor.matmul(ps_w[:, 0:128], wsrc[:, 0:128],
                                     wsrc[:, 128:256], start=True, stop=True,
                                     skip_group_check=True)
            warm(30)

            ps_re, ps_im = {}, {}
            for bt in range(2):
                ps_re[bt] = ps_pool.tile([128, I_SHARD], f32, tag="psre",
                                         name=f"psre{bt}")
                ps_im[bt] = ps_pool.tile([128, I_SHARD], f32, tag="psim",
                                         name=f"psim{bt}")
            # re = P @ Ur ; im = P @ Ui  (P real, stationary per (jt, bt)).
            # Keep-warm fillers between chunk groups: matmuls for the next
            # chunk wait on its DMA, and a >~0.3us PE gap re-throttles the
            # clock; fillers idle-spin the PE across the expected gaps.
            # Sized generously: an unneeded filler costs 56ns, but a PE gap
            # re-throttles the clock and costs ~1.2us on the following mms.
            fillers = {1: 4, 2: 10, 4: 4}
            for jt in range(NJ):
                warm(fillers.get(jt, 0))
                kj = 89 if jt == NJ - 1 else 128
                ur = c_t[jt][:kj, B_SHARD:B_SHARD + I_SHARD]
                ui = c_t[jt][:kj, B_SHARD + I_SHARD:B_SHARD + 2 * I_SHARD]
                for bt in range(2):
                    pj = c_t[jt][:kj, bt * 128:(bt + 1) * 128]
                    first, last = jt == 0, jt == NJ - 1
                    nc.tensor.matmul(ps_re[bt], pj, ur,
                                     start=first, stop=last)
                    nc.tensor.matmul(ps_im[bt], pj, ui,
                                     start=first, stop=last)
            # psum -> sbuf casts split across vector (re) and scalar (im);
            # each half-batch output rides its own HWDGE ring.
            for bt in range(2):
                sb = o_pool.tile([128, UB], bf16, tag="sb", name=f"sb{bt}")
                nc.vector.tensor_copy(out=sb[:, :I_SHARD], in_=ps_re[bt])
                nc.scalar.copy(out=sb[:, I_SHARD:], in_=ps_im[bt])
                (nc.sync if bt == 0 else nc.scalar).dma_start(
                    out=o_ri[bt * 128:(bt + 1) * 128], in_=sb)
    nc.compile()
    return nc


def kernel(x, theta_1, theta_2, squeezing_r, displacement_r, kerr_params):
    global LAST_RESULT
    import ml_dtypes
    bf16 = ml_dtypes.bfloat16

    x = np.asarray(x, dtype=np.float32)
    UT = _build_UT(np.asarray(theta_1, np.float64), np.asarray(theta_2, np.float64),
                   np.asarray(squeezing_r, np.float64),
                   np.asarray(displacement_r, np.float64),
                   np.asarray(kerr_params, np.float64))
    psi0 = _build_psi0(x.astype(np.float64))          # (B, 729) real
    p_t = psi0.T                                      # (729, B)

    UT_pad = np.zeros((DIM_PAD, DIM), np.complex128)
    UT_pad[:DIM] = UT
    p_pad = np.zeros((DIM_PAD, BATCH), np.float64)
    p_pad[:DIM] = p_t

    def pack_c(parr, uarr):
        """[768,256] real p + [768,366] complex u -> [128, 6*988] bf16:
        per j block [p (256) | u_re (366) | u_im (366)]."""
        CB = B_SHARD + 2 * I_SHARD
        pb = parr.reshape(NJ, 128, B_SHARD)
        ub = uarr.reshape(NJ, 128, I_SHARD)
        out = np.empty((128, NJ, CB), np.float32)
        out[:, :, :B_SHARD] = pb.transpose(1, 0, 2)
        out[:, :, B_SHARD:B_SHARD + I_SHARD] = ub.real.transpose(1, 0, 2)
        out[:, :, B_SHARD + I_SHARD:] = ub.imag.transpose(1, 0, 2)
        return np.ascontiguousarray(out.reshape(128, NJ * CB)).astype(bf16)

    in_maps = []
    for c in range(N_CORES):
        q, h = divmod(c, 2)
        bsl = slice(q * B_SHARD, (q + 1) * B_SHARD)
        isl = slice(I_START[h], I_START[h] + I_SHARD)
        in_maps.append({
            "c_ri": pack_c(p_pad[:, bsl], UT_pad[:, isl]),
        })

    # bass_utils' trace path does `from antenv.axon_hooks import ...`
    # unguarded; this image's antenv lacks that module.  Provide a stub so
    # tracing degrades gracefully instead of crashing (e.g. if BASS_TRACE=1).
    try:
        import antenv.axon_hooks  # noqa: F401
    except ImportError:
        import sys
        import types
        stub = types.ModuleType("antenv.axon_hooks")
        stub._hook = None
        stub.set_axon_ntff_profile_hook = lambda h: setattr(stub, "_hook", h)
        stub.get_axon_ntff_profile_hook = lambda: stub._hook
        sys.modules["antenv.axon_hooks"] = stub

    from concourse.bass_utils import run_bass_kernel_spmd
    nc = _build_bass()
    res = run_bass_kernel_spmd(nc, in_maps, core_ids=list(range(N_CORES)),
                               trace=bool(int(os.environ.get("KERNEL_TRACE", "0"))))
    LAST_RESULT = res

    psi = np.empty((BATCH, DIM), dtype=np.complex128)
    for c in range(N_CORES):
        q, h = divmod(c, 2)
        o = np.asarray(res.results[c]["o_ri"], dtype=np.float64)
        sh = o[:, :I_SHARD] + 1j * o[:, I_SHARD:]
        bsl = slice(q * B_SHARD, (q + 1) * B_SHARD)
        if h == 0:
            psi[bsl, 0:I_SHARD] = sh
        else:
            psi[bsl, I_SHARD:DIM] = sh[:, I_SHARD - (DIM - I_SHARD):]
    return _expectation(psi).astype(np.float32)


# revision 23
# speedup vs baseline: 1.0496x; 1.0496x over previous
"""CV neural network (6 modes, cutoff 3, 6 layers) on 8 trn2 NeuronCores.

Algebra: the reference circuit is
    psi0(x_b) = kron_m expm(x_bm * D_GEN)[:, 0]          (closed form, host)
    psi       = C @ psi0                                  (C fixed 729x729)
    out[b,m]  = Re( psi^H (I (x) X_OP (x) I) psi )        (host)
Everything between the data-encoding displacements and the expectations is a
fixed linear operator C on the 729-dim truncated Fock space, depending only on
the (tiny) layer parameters.  The host folds the circuit into UT = C^T once
(complex128); the device does the heavy part: psi[b,:] = psi0[b,:] @ UT.

Key structure: D_GEN = A^T - A is REAL, so the encoding displacement gates and
hence psi0 are REAL vectors.  The device matmul is therefore real x complex:
    re = P @ Ur,  im = P @ Ui            (2 real matmuls, not 4)
and with the loose 2e-2 rel-err budget all operands ride in bf16 (measured
end-to-end rel err ~1.6e-3), halving DMA bytes again vs f32.

Sharding: batch 4-way x output-column (i) 2-way = 8 cores.  Per core:
  p_ri: [128, 6*256]   psi0^T batch-quarter, bf16, K-blocked (j = 128*jt + p)
  u_ri: [128, 6*732]   UT column half, bf16, K-blocked, each block re|im
  o_ri: [256, 732]     psi shard bf16 (b rows; cols re|im)
Five DMAs total (p, u in 3 chunks, out x2), all on the two HWDGE rings; no
SWDGE, no scalar-engine compute (avoids its ACT table load).
"""
import os
import numpy as np

N_MODES, N_LAYERS, CUTOFF, BATCH = 6, 6, 3, 1024
M2 = N_MODES * (N_MODES - 1) // 2
DIM = CUTOFF ** N_MODES                      # 729
N_CORES = 8
B_SHARD = BATCH // 4                         # 256 (batch quarter)
I_SHARD = 366                                # column half; overlap 3
I_START = (0, DIM - I_SHARD)                 # (0, 363)
DIM_PAD = 768                                # 6 x 128 (rows 729.. are zero)
NJ = 6                                       # j tiles, K=128 (last 89 live)

# Results of the last device run (for the test harness to inspect).
LAST_RESULT = None

# ----------------------------------------------------------------- host math

_a = np.diag(np.sqrt(np.arange(1, CUTOFF)), 1).astype(np.complex128)
_ad = _a.conj().T
_NVEC = np.arange(CUTOFF, dtype=np.float64)
_X_OP = (_a + _ad).real
_BS_GEN = np.kron(_ad, _a) - np.kron(_a, _ad)
_SQ_GEN = _a @ _a - _ad @ _ad
_D_GEN = _ad - _a


def _expm_factory(G):
    """G anti-Hermitian. Returns f(t) = expm(t*G), vectorized over real t."""
    lam, V = np.linalg.eigh(1j * G)
    Vh = V.conj().T

    def f(t):
        t = np.asarray(t, dtype=np.float64)
        ph = np.exp(-1j * np.multiply.outer(t, lam))
        return np.einsum('ij,...j,jk->...ik', V, ph, Vh)
    return f


_disp_gate = _expm_factory(_D_GEN)
_sq_gate_half = _expm_factory(0.5 * _SQ_GEN)
_bs_gate = _expm_factory(_BS_GEN)


def _apply_1(psi, U, m):
    psi = np.moveaxis(psi, 1 + m, -1)
    psi = psi @ U.T
    return np.moveaxis(psi, -1, 1 + m)


def _apply_2(psi, U, m):
    psi = np.moveaxis(psi, (1 + m, 2 + m), (-2, -1))
    sh = psi.shape
    psi = (psi.reshape(sh[:-2] + (CUTOFF * CUTOFF,)) @ U.T).reshape(sh)
    return np.moveaxis(psi, (-2, -1), (1 + m, 2 + m))


def _apply_diag(psi, d, m):
    shape = [1] * psi.ndim
    shape[1 + m] = CUTOFF
    return psi * d.reshape(shape)


def _interferometer(psi, params):
    theta = params[:M2]
    rphi = params[-N_MODES:]
    n = 0
    for l in range(N_MODES):
        for k in range(N_MODES - 1):
            if (l + k) % 2 != 1:
                psi = _apply_2(psi, _bs_gate(theta[n]), k)
                n += 1
    for i in range(max(1, N_MODES - 1)):
        psi = _apply_diag(psi, np.exp(1j * rphi[i] * _NVEC), i)
    return psi


def _build_UT(theta_1, theta_2, squeezing_r, displacement_r, kerr_params):
    """UT[j, i] = C[i, j]: apply the post-encoding circuit to basis vectors."""
    psi = np.eye(DIM, dtype=np.complex128).reshape((DIM,) + (CUTOFF,) * N_MODES)
    for L in range(N_LAYERS):
        psi = _interferometer(psi, theta_1[L])
        for m in range(N_MODES):
            psi = _apply_1(psi, _sq_gate_half(squeezing_r[L, m] * 0.5), m)
        psi = _interferometer(psi, theta_2[L])
        for m in range(N_MODES):
            psi = _apply_1(psi, _disp_gate(displacement_r[L, m]), m)
            psi = _apply_diag(
                psi, np.exp(1j * (kerr_params[L, m] * 0.001) * _NVEC * _NVEC), m)
    return psi.reshape(DIM, DIM)


def _build_psi0(x):
    """x: (B, 6) -> flattened kron of displacement columns, (B, 729) real."""
    v = _disp_gate(x)[..., :, 0].real
    out = v[:, 0, :]
    for m in range(1, N_MODES):
        out = np.einsum('bi,bj->bij', out, v[:, m, :]).reshape(x.shape[0], -1)
    return out


def _expectation(psi_flat):
    """psi_flat: (B, 729) complex -> (B, 6) float64: <X_m>."""
    B = psi_flat.shape[0]
    outs = []
    for m in range(N_MODES):
        pre, post = CUTOFF ** m, CUTOFF ** (N_MODES - 1 - m)
        psi = psi_flat.reshape(B, pre, CUTOFF, post)
        phi = np.einsum('ij,bpjq->bpiq', _X_OP, psi)
        outs.append(np.sum(psi.conj() * phi, axis=(1, 2, 3)).real)
    return np.stack(outs, axis=1)


# --------------------------------------------------------------- bass kernel

def _build_bass():
    import concourse.mybir as mybir
    import concourse.tile as tile
    from concourse import bacc

    nc = bacc.Bacc("TRN2", target_bir_lowering=False, debug=False,
                   enable_asserts=False, num_devices=N_CORES)
    f32 = mybir.dt.float32
    bf16 = mybir.dt.bfloat16

    UB = 2 * I_SHARD                               # 732: re|im per j block
    CB = B_SHARD + UB                              # 988 cols per jt bundle
    c_ri = nc.dram_tensor("c_ri", [128, NJ * CB], bf16,
                          kind="ExternalInput").ap()
    o_ri = nc.dram_tensor("o_ri", [B_SHARD, UB], bf16,
                          kind="ExternalOutput").ap()

    with tile.TileContext(nc) as tc:
        with (
            tc.tile_pool(name="c", bufs=6) as c_pool,
            tc.tile_pool(name="ps", bufs=2, space="PSUM") as ps_pool,
            tc.tile_pool(name="o", bufs=2) as o_pool,
            tc.tile_pool(name="s", bufs=1) as s_pool,
        ):
            # Bundles [p jt | u_re jt | u_im jt].  Ring facts (measured):
            # each HWDGE ring moves ~160 GB/s with >=2.9KB/partition
            # descriptors but only ~110-130 with ~2KB ones; rings drain
            # FIFO; chunks on different rings round-robin and complete
            # together.  So: small single-jt chunks first on each ring for
            # an early start, big 2-jt chunks behind them for rate; equal
            # bytes per ring.  128 rows always (an 89-row DMA degenerates to
            # a serial ~77ns/descriptor trickle).
            # Warm-tile memset first in the gpsimd stream so PE warm-up can
            # start at ~7.4us (vector's first op otherwise lands ~1us late).
            wsrc = s_pool.tile([128, 256], bf16, tag="warm", name="warm")
            nc.gpsimd.memset(wsrc[:, :], 0)

            groups = [((0,), nc.sync), ((1,), nc.scalar),
                      ((2, 3), nc.sync), ((4, 5), nc.scalar)]
            c_t = {}
            for g, (jts, eng) in enumerate(groups):
                t = c_pool.tile([128, len(jts) * CB], bf16, tag="c",
                                name=f"c{g}")
                eng.dma_start(
                    out=t, in_=c_ri[:, jts[0] * CB:(jts[-1] + 1) * CB])
                for i, jt in enumerate(jts):
                    c_t[jt] = t[:, i * CB:(i + 1) * CB]

            # PE warm-up: back-to-back dummy bf16 matmuls (~100% PE duty)
            # bridge the input-load window; the HAM un-throttles the PE
            # (1.2 -> 2.4 GHz) only after ~3.8us of gapless activity, and any
            # idle gap restarts it cold.
            ps_w = ps_pool.tile([128, 512], f32, tag="psw", name="psw", bufs=1)

            def warm(n):
                for w in range(n):
                    nc.tensor.matmul(ps_w[:, 0:128], wsrc[:, 0:128],
                                     wsrc[:, 128:256], start=True, stop=True,
                                     skip_group_check=True)
            warm(30)

            ps_re, ps_im = {}, {}
            for bt in range(2):
                ps_re[bt] = ps_pool.tile([128, I_SHARD], f32, tag="psre",
                                         name=f"psre{bt}")
                ps_im[bt] = ps_pool.tile([128, I_SHARD], f32, tag="psim",
                                         name=f"psim{bt}")
            # re = P @ Ur ; im = P @ Ui  (P real, stationary per (jt, bt)).
            # Keep-warm fillers between chunk groups: matmuls for the next
            # chunk wait on its DMA, and a >~0.3us PE gap re-throttles the
            # clock; fillers idle-spin the PE across the expected gaps.
            # Sized generously: an unneeded filler costs 56ns, but a PE gap
            # re-throttles the clock and costs ~1.2us on the following mms.
            fillers = {1: 4, 2: 6, 4: 3}
            for jt in range(NJ):
                warm(fillers.get(jt, 0))
                kj = 89 if jt == NJ - 1 else 128
                ur = c_t[jt][:kj, B_SHARD:B_SHARD + I_SHARD]
                ui = c_t[jt][:kj, B_SHARD + I_SHARD:B_SHARD + 2 * I_SHARD]
                for bt in range(2):
                    pj = c_t[jt][:kj, bt * 128:(bt + 1) * 128]
                    first, last = jt == 0, jt == NJ - 1
                    nc.tensor.matmul(ps_re[bt], pj, ur,
                                     start=first, stop=last)
                    nc.tensor.matmul(ps_im[bt], pj, ui,
                                     start=first, stop=last)
            # psum -> sbuf casts split across vector (re) and scalar (im);
            # each half-batch output rides its own HWDGE ring.
            for bt in range(2):
                sb = o_pool.tile([128, UB], bf16, tag="sb", name=f"sb{bt}")
                nc.vector.tensor_copy(out=sb[:, :I_SHARD], in_=ps_re[bt])
                nc.scalar.copy(out=sb[:, I_SHARD:], in_=ps_im[bt])
                (nc.sync if bt == 0 else nc.scalar).dma_start(
                    out=o_ri[bt * 128:(bt + 1) * 128], in_=sb)
    nc.compile()
    return nc


def kernel(x, theta_1, theta_2, squeezing_r, displacement_r, kerr_params):
    global LAST_RESULT
    import ml_dtypes
    bf16 = ml_dtypes.bfloat16

    x = np.asarray(x, dtype=np.float32)
    UT = _build_UT(np.asarray(theta_1, np.float64), np.asarray(theta_2, np.float64),
                   np.asarray(squeezing_r, np.float64),
                   np.asarray(displacement_r, np.float64),
                   np.asarray(kerr_params, np.float64))
    psi0 = _build_psi0(x.astype(np.float64))          # (B, 729) real
    p_t = psi0.T                                      # (729, B)

    UT_pad = np.zeros((DIM_PAD, DIM), np.complex128)
    UT_pad[:DIM] = UT
    p_pad = np.zeros((DIM_PAD, BATCH), np.float64)
    p_pad[:DIM] = p_t

    def pack_c(parr, uarr):
        """[768,256] real p + [768,366] complex u -> [128, 6*988] bf16:
        per j block [p (256) | u_re (366) | u_im (366)]."""
        CB = B_SHARD + 2 * I_SHARD
        pb = parr.reshape(NJ, 128, B_SHARD)
        ub = uarr.reshape(NJ, 128, I_SHARD)
        out = np.empty((128, NJ, CB), np.float32)
        out[:, :, :B_SHARD] = pb.transpose(1, 0, 2)
        out[:, :, B_SHARD:B_SHARD + I_SHARD] = ub.real.transpose(1, 0, 2)
        out[:, :, B_SHARD + I_SHARD:] = ub.imag.transpose(1, 0, 2)
        return np.ascontiguousarray(out.reshape(128, NJ * CB)).astype(bf16)

    in_maps = []
    for c in range(N_CORES):
        q, h = divmod(c, 2)
        bsl = slice(q * B_SHARD, (q + 1) * B_SHARD)
        isl = slice(I_START[h], I_START[h] + I_SHARD)
        in_maps.append({
            "c_ri": pack_c(p_pad[:, bsl], UT_pad[:, isl]),
        })

    # bass_utils' trace path does `from antenv.axon_hooks import ...`
    # unguarded; this image's antenv lacks that module.  Provide a stub so
    # tracing degrades gracefully instead of crashing (e.g. if BASS_TRACE=1).
    try:
        import antenv.axon_hooks  # noqa: F401
    except ImportError:
        import sys
        import types
        stub = types.ModuleType("antenv.axon_hooks")
        stub._hook = None
        stub.set_axon_ntff_profile_hook = lambda h: setattr(stub, "_hook", h)
        stub.get_axon_ntff_profile_hook = lambda: stub._hook
        sys.modules["antenv.axon_hooks"] = stub

    from concourse.bass_utils import run_bass_kernel_spmd
    nc = _build_bass()
    res = run_bass_kernel_spmd(nc, in_maps, core_ids=list(range(N_CORES)),
                               trace=bool(int(os.environ.get("KERNEL_TRACE", "0"))))
    LAST_RESULT = res

    psi = np.empty((BATCH, DIM), dtype=np.complex128)
    for c in range(N_CORES):
        q, h = divmod(c, 2)
        o = np.asarray(res.results[c]["o_ri"], dtype=np.float64)
        sh = o[:, :I_SHARD] + 1j * o[:, I_SHARD:]
        bsl = slice(q * B_SHARD, (q + 1) * B_SHARD)
        if h == 0:
            psi[bsl, 0:I_SHARD] = sh
        else:
            psi[bsl, I_SHARD:DIM] = sh[:, I_SHARD - (DIM - I_SHARD):]
    return _expectation(psi).astype(np.float32)
